# revision 46
# baseline (speedup 1.0000x reference)
"""Trainium2 Bass kernel for nn_HNC_strategy (hypernetwork-conditioned MLP).

Math (per sample b):
  A[b,:]   = tanh-MLP hypernet of [t-0.5, freqs[b]]          -> [8]
  x[b,:]   = [cos(y[b,:64]), sin(y[b,:64])]                  -> [128]
  W1[b]    = sum_k A[b,k] w1k  (+ head_b part)               -> [65,128]
  hid      = tanh(W1[b] @ x[b] + b1[b])                      -> [65]
  out      = W2[b] @ hid + b2[b]                             -> [64]

Strategy: G_k = x @ w1k^T computed as shared bf16 matmuls (batch on
partition, fp32 PSUM accumulate); the per-sample k-contraction
sum_k A[b,k]*G_k[b,:] reads PSUM directly: a DVE STT chain covers most
k-terms, ACT scaled-copies produce the rest, and Pool (gpsimd) does the
SBUF-only add tree. Engine-ISA constraints found on real TRN2: gpsimd
cannot access PSUM and TensorScalarPtr is not a valid Pool opcode, so
every scalar MAC must live on DVE or ACT — Pool can only tensor-add.
head_w's bias blocks are folded into the G matmuls via ones-rows
(xA row 64 = 1 from host; hid col 65 = tanh(37) = 1 via preset psum-acc
column), so no separate bias matmuls are needed. Trig features are
computed host-side in float64 and shipped as bf16 (the bf16 matmul
would quantize them identically anyway). All weights ship in one packed
blob: DMA trigger descriptor-generation costs ~0.5us per transfer on the
issuing engine, so fewer, bigger DMAs win.

Sharding: pure data parallel over 8 NeuronCores (2048 rows each).
"""

import sys

sys.path.insert(0, "/opt/trn_rl_repo")

import numpy as np

import concourse.bacc as bacc
import concourse.mybir as mybir
import concourse.tile as tile
from concourse.alu_op_type import AluOpType
from concourse.bass_utils import run_bass_kernel_spmd

DIM = 64
MLPS = DIM + 1          # 65
B = 16384
H = DIM + 2             # 66
P = MLPS * 2 * DIM + MLPS + DIM * MLPS + DIM
O1 = MLPS * 2 * DIM     # 8320  end of W1 block
O2 = O1 + MLPS          # 8385  end of b1 block
O3 = O2 + DIM * MLPS    # 12545 end of W2 block
N_CORES = 8
BS = B // N_CORES       # 2048 rows per core
CH = 512                # phase-A chunk
NCH = BS // CH          # 4
NT = BS // 128          # 16 batch tiles per core

# packed weight blob column offsets (bf16, [128, XB])
B_W0T = 0
B_W1T = B_W0T + H        # 66
B_W2T = B_W1T + H        # 132
B_W3T = B_W2T + H        # 198
B_W1RA = B_W3T + 8       # 206
B_W1RB = B_W1RA + 8 * MLPS   # 726
B_W2RE = B_W1RB + 8 * MLPS   # 1246
B_IDENT = B_W2RE + 8 * DIM   # 1758
XB0 = B_IDENT + 128          # 1886
B_C1FA = XB0                 # flag-only regions
B_C1FB = B_C1FA + MLPS
B_C2F = B_C1FB + MLPS
XB1 = B_C2F + DIM            # 2080

F32 = mybir.dt.float32
BF16 = mybir.dt.bfloat16
NPBF = mybir.dt.np(BF16)
TANH = mybir.ActivationFunctionType.Tanh
COPY = mybir.ActivationFunctionType.Copy

_CACHE: dict = {}


def build_bass(has_hbw: bool, has_hb3: bool):
    nc = bacc.Bacc("TRN2", target_bir_lowering=False, debug=False,
                   num_devices=N_CORES)

    XB = XB1 if has_hbw else XB0
    d_zT = nc.dram_tensor("zT", [MLPS, BS], BF16, kind="ExternalInput")
    d_xA = nc.dram_tensor("xA", [MLPS, BS], BF16, kind="ExternalInput")
    d_xB = nc.dram_tensor("xB", [DIM, BS], BF16, kind="ExternalInput")
    d_wb = nc.dram_tensor("wb", [128, XB], BF16, kind="ExternalInput")
    d_fb = nc.dram_tensor("fb", [H, 3], F32, kind="ExternalInput")
    d_c37 = nc.dram_tensor("c37", [128, 1], F32, kind="ExternalInput")
    d_hb3r = (nc.dram_tensor("hb3r", [128, 64], F32, kind="ExternalInput")
              if has_hb3 else None)
    d_out = nc.dram_tensor("out", [BS, DIM], F32, kind="ExternalOutput")

    mult, add = AluOpType.mult, AluOpType.add
    PSUM = "PSUM"

    with tile.TileContext(nc) as tc:
        with tc.tile_pool(name="const", bufs=1) as cp:
            zT = cp.tile([MLPS, BS], BF16)
            xA = cp.tile([MLPS, BS], BF16)
            xB = cp.tile([DIM, BS], BF16)
            wb = cp.tile([128, XB], BF16)
            fb = cp.tile([H, 3], F32)
            c37 = cp.tile([128, 1], F32)
            hb3r = cp.tile([128, 64], F32) if has_hb3 else None

            w0T = wb[0:MLPS, B_W0T:B_W0T + H]
            w1T = wb[0:H, B_W1T:B_W1T + H]
            w2T = wb[0:H, B_W2T:B_W2T + H]
            w3T = wb[0:H, B_W3T:B_W3T + 8]
            w1rA = wb[0:MLPS, B_W1RA:B_W1RA + 8 * MLPS]
            w1rB = wb[0:DIM, B_W1RB:B_W1RB + 8 * MLPS]
            w2re = wb[0:H, B_W2RE:B_W2RE + 8 * DIM]
            ident = wb[0:128, B_IDENT:B_IDENT + 128]
            hb0, hb1, hb2 = fb[:, 0:1], fb[:, 1:2], fb[:, 2:3]
            c1fA = wb[0:MLPS, B_C1FA:B_C1FA + MLPS] if has_hbw else None
            c1fB = wb[0:DIM, B_C1FB:B_C1FB + MLPS] if has_hbw else None
            c2f = wb[0:H, B_C2F:B_C2F + DIM] if has_hbw else None

            # input loads: 3 parallel DMA queues; per-transfer trigger cost
            # is ~0.5-0.8us on the issuing engine, so few big transfers.
            # Phase-A weights (blob cols 0:B_W1RA) ship first and tiny.
            HB = BS // 2
            nc.scalar.dma_start(zT[:, 0:HB], d_zT[:, 0:HB])
            nc.scalar.dma_start(wb[:, 0:B_W1RA], d_wb[:, 0:B_W1RA])
            nc.sync.dma_start(wb[:, B_W1RA:B_W2RE], d_wb[:, B_W1RA:B_W2RE])
            nc.sync.dma_start(wb[:, B_W2RE:XB], d_wb[:, B_W2RE:XB])
            nc.sync.dma_start(fb[:], d_fb[:])
            nc.sync.dma_start(c37[:], d_c37[:])
            nc.sync.dma_start(zT[:, HB:BS], d_zT[:, HB:BS])
            nc.sync.dma_start(xA[:, HB:BS], d_xA[:, HB:BS])
            nc.sync.dma_start(xB[:, HB:BS], d_xB[:, HB:BS])
            if has_hb3:
                nc.sync.dma_start(hb3r[:], d_hb3r[:])
            nc.gpsimd.dma_start(xA[:, 0:HB], d_xA[:, 0:HB])
            nc.gpsimd.dma_start(xB[:, 0:HB], d_xB[:, 0:HB])

            Ab = cp.tile([128, 8 * NT], F32)     # batch-major A scalars
            accs = [cp.tile([128, H], F32, name=f"acc{i}") for i in range(4)]
            for a_ in accs:
                nc.vector.tensor_copy(a_[:, MLPS:H], c37[:])  # tanh(37)==1

            with (
                tc.tile_pool(name="ha", bufs=2) as ha,
                tc.tile_pool(name="psA", bufs=1, space=PSUM) as psA,
                tc.tile_pool(name="pk", bufs=3, space=PSUM) as pk,
                tc.tile_pool(name="g2", bufs=2, space=PSUM) as g2,
                tc.tile_pool(name="tp", bufs=1, space=PSUM) as tp,
                tc.tile_pool(name="pb", bufs=1, space=PSUM) as pb,
                tc.tile_pool(name="hd", bufs=3) as hd,
                tc.tile_pool(name="ht", bufs=3) as ht,
                tc.tile_pool(name="ob", bufs=7) as ob,
            ):
                # ---- phase A: hypernet (feature-on-partition, bf16) ----
                # emitted layer-by-layer, interleaved with main-loop work:
                # PE queues are in-order, so a monolithic mm->tanh->mm chain
                # would stall every matmul queued behind it.
                pa_state: dict = {}

                def pa_step(c, step):
                    if step == 0:
                        sl = slice(c * CH, (c + 1) * CH)
                        p0 = psA.tile([H, CH], F32, tag="hp")
                        nc.tensor.matmul(p0[:], w0T, zT[:, sl])
                        h0 = ha.tile([H, CH], BF16, tag="h")
                        nc.scalar.activation(h0[:], p0[:], TANH, bias=hb0)
                        pa_state[c] = h0
                    elif step == 1:
                        p1 = psA.tile([H, CH], F32, tag="hp")
                        nc.tensor.matmul(p1[:], w1T, pa_state[c][:])
                        h1 = ha.tile([H, CH], BF16, tag="h")
                        nc.scalar.activation(h1[:], p1[:], TANH, bias=hb1)
                        pa_state[c] = h1
                    elif step == 2:
                        p2 = psA.tile([H, CH], F32, tag="hp")
                        nc.tensor.matmul(p2[:], w2T, pa_state[c][:])
                        h2 = ha.tile([H, CH], BF16, tag="h")
                        nc.scalar.activation(h2[:], p2[:], TANH, bias=hb2)
                        pa_state[c] = h2
                    else:
                        h2 = pa_state.pop(c)
                        pAb = pb.tile([128, 4 * 8], F32, tag="pAb")
                        for j4 in range(4):
                            nc.tensor.matmul(pAb[:, j4 * 8:(j4 + 1) * 8],
                                             h2[:, j4 * 128:(j4 + 1) * 128],
                                             w3T)
                        if has_hb3:
                            nc.vector.tensor_add(pAb[:], pAb[:],
                                                 hb3r[:, 0:4 * 8])
                        nc.scalar.activation(Ab[:, c * 32:(c + 1) * 32],
                                             pAb[:], TANH)

                # ---- main loop, software-pipelined ----
                # stage G: G1 matmuls + cast-copy to SBUF bf16 (DVE)
                # stage C: Pool L1 chain, tanh, transpose, G2, split copy,
                #          Pool L2 chain, out-DMA
                g1sbs = {}

                def stage_g(j):
                    jsl = slice(j * 128, (j + 1) * 128)
                    pA = pk.tile([128, 4 * MLPS], F32, tag="pk")
                    nc.tensor.matmul(pA[:], xA[:, jsl], w1rA[:, 0:4 * MLPS],
                                     start=True, stop=False)
                    nc.tensor.matmul(pA[:], xB[:, jsl], w1rB[:, 0:4 * MLPS],
                                     start=False, stop=True)
                    pB = pk.tile([128, 4 * MLPS], F32, tag="pk")
                    nc.tensor.matmul(pB[:], xA[:, jsl],
                                     w1rA[:, 4 * MLPS:8 * MLPS],
                                     start=True, stop=False)
                    nc.tensor.matmul(pB[:], xB[:, jsl],
                                     w1rB[:, 4 * MLPS:8 * MLPS],
                                     start=False, stop=True)
                    g1sbs[j] = (pA, pB)

                def stage_c(j):
                    jsl = slice(j * 128, (j + 1) * 128)
                    sA = lambda k: Ab[:, j * 8 + k: j * 8 + k + 1]
                    acc = accs[j % 4]
                    am = acc[:, 0:MLPS]
                    pA, pB = g1sbs.pop(j)
                    gA = lambda k: pA[:, k * MLPS:(k + 1) * MLPS]
                    gB = lambda k: pB[:, k * MLPS:(k + 1) * MLPS]
                    # L1 direct from PSUM. TensorScalarPtr is NOT a valid
                    # Pool opcode on TRN2, so scalar MACs live on DVE (STT
                    # chain) and ACT (scaled copies); Pool does plain adds.
                    nc.vector.tensor_scalar(am, gA(0), sA(0), None, mult)
                    for k in range(1, 4):
                        nc.vector.scalar_tensor_tensor(
                            am, gA(k), sA(k), am, mult, add)
                    lc = [ob.tile([128, MLPS], F32, tag=f"lc{k}",
                                  name=f"lc{k}") for k in range(3)]
                    for k in range(3):
                        nc.scalar.activation(lc[k][:], gB(1 + k), COPY,
                                             bias=0.0, scale=sA(5 + k))
                    nc.vector.scalar_tensor_tensor(
                        am, gB(0), sA(4), am, mult, add)
                    u0 = ob.tile([128, MLPS], F32, tag="u0")
                    nc.gpsimd.tensor_tensor(u0[:], lc[0][:], lc[1][:], add)
                    nc.gpsimd.tensor_tensor(u0[:], u0[:], lc[2][:], add)
                    nc.gpsimd.tensor_tensor(am, am, u0[:], add)
                    if has_hbw:
                        c1p = tp.tile([128, MLPS], F32, tag="c1")
                        nc.tensor.matmul(c1p[:], xA[:, jsl], c1fA,
                                         start=True, stop=False)
                        nc.tensor.matmul(c1p[:], xB[:, jsl], c1fB,
                                         start=False, stop=True)
                        nc.vector.scalar_tensor_tensor(
                            am, c1p[:], 1.0, am, mult, add)

                    hid = hd.tile([128, H], BF16, tag="hid")
                    nc.scalar.activation(hid[:], acc[:], TANH)
                    tpp = tp.tile([H, 128], BF16, tag="tp")
                    nc.tensor.transpose(tpp[:], hid[:], ident)
                    hidT = ht.tile([H, 128], BF16, tag="hidT")
                    nc.vector.tensor_copy(hidT[:], tpp[:])

                    pG2 = g2.tile([128, 8 * DIM], F32, tag="g2")
                    nc.tensor.matmul(pG2[:], hidT[:], w2re)
                    # L2 direct from PSUM: DVE STT chain k0..k4, ACT
                    # scaled-copy products k5..k7, Pool add tree
                    g2s = lambda k: pG2[:, k * DIM:(k + 1) * DIM]
                    of = ob.tile([128, DIM], F32, tag="of")
                    nc.vector.tensor_scalar(of[:], g2s(0), sA(0), None,
                                            mult)
                    for k in range(1, 5):
                        nc.vector.scalar_tensor_tensor(
                            of[:], g2s(k), sA(k), of[:], mult, add)
                    oc = [ob.tile([128, DIM], F32, tag=f"oc{k}",
                                  name=f"oc{k}") for k in range(3)]
                    for i, k in enumerate(range(5, 8)):
                        nc.scalar.activation(oc[i][:], g2s(k), COPY,
                                             bias=0.0, scale=sA(k))
                    t0 = ob.tile([128, DIM], F32, tag="t0")
                    nc.gpsimd.tensor_tensor(t0[:], oc[0][:], oc[1][:], add)
                    nc.gpsimd.tensor_tensor(t0[:], t0[:], oc[2][:], add)
                    nc.gpsimd.tensor_tensor(of[:], of[:], t0[:], add)
                    if has_hbw:
                        c2p = tp.tile([128, DIM], F32, tag="c2")
                        nc.tensor.matmul(c2p[:], hidT[:], c2f)
                        nc.vector.scalar_tensor_tensor(
                            of[:], c2p[:], 1.0, of[:], mult, add)
                    nc.sync.dma_start(d_out[jsl, :], of[:])

                # emission order = per-engine queue order: phase-A steps are
                # spread between stage_g/stage_c emissions so no engine's
                # in-order queue ever parks on a cross-engine round-trip
                LOOKAHEAD = 5
                pa_step(0, 0)
                pa_step(1, 0)
                stage_g(0)
                pa_step(0, 1)
                stage_g(1)
                pa_step(0, 2)
                stage_g(2)
                pa_step(0, 3)          # Ab chunk 0 (tiles 0-3) ready
                stage_g(3)
                pa_step(1, 1)
                stage_g(4)
                pa_step(1, 2)
                pa_step(1, 3)          # Ab chunk 1 (tiles 4-7)
                pa_plan = {2: (2, 0), 3: (2, 1), 4: (2, 2), 5: (2, 3),
                           6: (3, 0), 7: (3, 1), 8: (3, 2), 9: (3, 3)}
                for j in range(NT):
                    if j in pa_plan:
                        pa_step(*pa_plan[j])
                    if j + LOOKAHEAD < NT:
                        stage_g(j + LOOKAHEAD)
                    stage_c(j)

    nc.compile()
    return nc


def _prep(inputs):
    f = lambda name: np.ascontiguousarray(
        np.asarray(inputs[name], dtype=np.float32))
    t = float(np.asarray(inputs["t"]))
    y, freqs = f("y"), f("freqs")
    hw0, hb0 = f("hw0"), f("hb0")
    hw1, hb1 = f("hw1"), f("hb1")
    hw2, hb2 = f("hw2"), f("hb2")
    hw3, hb3 = f("hw3"), f("hb3")
    head_w, head_b = f("head_w"), f("head_b")

    C = np.ascontiguousarray

    zT = np.empty((MLPS, B), np.float32)
    zT[0, :] = t - 0.5
    zT[1:, :] = freqs.T

    y64 = y[:, :DIM].astype(np.float64)
    xA = np.empty((MLPS, B), np.float32)
    xA[:DIM, :] = np.cos(y64).T
    xA[DIM, :] = 1.0
    xB = np.asarray(np.sin(y64).T, np.float32)

    # w1 fold: head_w W1-block + b1-w row, split cos/sin feature halves
    hw1b = head_w[:O1].reshape(MLPS, 2 * DIM, 8)        # [h, i, k]
    hw2b = head_w[O2:O3].reshape(DIM, MLPS, 8)          # [o, h, k]

    has_hbw = bool(np.any(head_b))
    has_hb3 = bool(np.any(hb3))
    XB = XB1 if has_hbw else XB0

    wb = np.zeros((128, XB), np.float32)
    wb[0:MLPS, B_W0T:B_W0T + H] = hw0.T
    wb[0:H, B_W1T:B_W1T + H] = hw1.T
    wb[0:H, B_W2T:B_W2T + H] = hw2.T
    wb[0:H, B_W3T:B_W3T + 8] = hw3.T
    wb[0:DIM, B_W1RA:B_W1RA + 8 * MLPS] = \
        hw1b[:, :DIM, :].transpose(1, 2, 0).reshape(DIM, 8 * MLPS)
    wb[DIM, B_W1RA:B_W1RA + 8 * MLPS] = head_w[O1:O2].T.reshape(8 * MLPS)
    wb[0:DIM, B_W1RB:B_W1RB + 8 * MLPS] = \
        hw1b[:, DIM:, :].transpose(1, 2, 0).reshape(DIM, 8 * MLPS)
    wb[0:MLPS, B_W2RE:B_W2RE + 8 * DIM] = \
        hw2b.transpose(1, 2, 0).reshape(MLPS, 8 * DIM)
    wb[MLPS, B_W2RE:B_W2RE + 8 * DIM] = head_w[O3:].T.reshape(8 * DIM)
    wb[0:128, B_IDENT:B_IDENT + 128] = np.eye(128, dtype=np.float32)
    if has_hbw:
        hb1b = head_b[:O1].reshape(MLPS, 2 * DIM)       # [h, i]
        wb[0:DIM, B_C1FA:B_C1FA + MLPS] = hb1b[:, :DIM].T
        wb[DIM, B_C1FA:B_C1FA + MLPS] = head_b[O1:O2]
        wb[0:DIM, B_C1FB:B_C1FB + MLPS] = hb1b[:, DIM:].T
        wb[0:MLPS, B_C2F:B_C2F + DIM] = head_b[O2:O3].reshape(DIM, MLPS).T
        wb[MLPS, B_C2F:B_C2F + DIM] = head_b[O3:]

    fbv = np.stack([hb0, hb1, hb2], axis=1).astype(np.float32)

    shared = {"wb": C(wb).astype(NPBF), "fb": C(fbv),
              "c37": np.full((128, 1), 37.0, np.float32)}
    if has_hb3:
        shared["hb3r"] = C(np.tile(hb3[None, :], (128, 8)))

    zTb = C(zT).astype(NPBF)
    xAb = C(xA).astype(NPBF)
    xBb = C(xB).astype(NPBF)
    in_maps = []
    for c in range(N_CORES):
        sl = slice(c * BS, (c + 1) * BS)
        in_maps.append({
            **shared,
            "zT": C(zTb[:, sl]),
            "xA": C(xAb[:, sl]),
            "xB": C(xBb[:, sl]),
        })
    return in_maps, (has_hbw, has_hb3)


def _run(inputs, trace=False):
    in_maps, flags = _prep(inputs)
    if flags not in _CACHE:
        _CACHE[flags] = build_bass(*flags)
    nc = _CACHE[flags]
    res = run_bass_kernel_spmd(nc, in_maps, core_ids=list(range(N_CORES)),
                               trace=trace)
    out = np.concatenate([r["out"] for r in res.results], axis=0)
    return out, res


def kernel(**inputs) -> np.ndarray:
    out, _ = _run(inputs)
    return out


if __name__ == "__main__":
    rng = np.random.default_rng(0)
    demo = {
        "t": np.float32(0.3),
        "y": rng.standard_normal((B, 2 * DIM), dtype=np.float32),
        "freqs": rng.random((B, DIM), dtype=np.float32),
        "hw0": rng.standard_normal((H, 1 + DIM), dtype=np.float32) * 0.05,
        "hb0": np.zeros(H, np.float32),
        "hw1": rng.standard_normal((H, H), dtype=np.float32) * 0.05,
        "hb1": np.zeros(H, np.float32),
        "hw2": rng.standard_normal((H, H), dtype=np.float32) * 0.05,
        "hb2": np.zeros(H, np.float32),
        "hw3": rng.standard_normal((8, H), dtype=np.float32) * 0.05,
        "hb3": np.zeros(8, np.float32),
        "head_w": rng.standard_normal((P, 8), dtype=np.float32) * 0.05,
        "head_b": np.zeros(P, np.float32),
    }
    out = kernel(**demo)
    print("out", out.shape, out.dtype, float(np.abs(out).max()))


# revision 50
# speedup vs baseline: 1.0052x; 1.0052x over previous
"""Trainium2 Bass kernel for nn_HNC_strategy (hypernetwork-conditioned MLP).

Math (per sample b):
  A[b,:]   = tanh-MLP hypernet of [t-0.5, freqs[b]]          -> [8]
  x[b,:]   = [cos(y[b,:64]), sin(y[b,:64])]                  -> [128]
  W1[b]    = sum_k A[b,k] w1k  (+ head_b part)               -> [65,128]
  hid      = tanh(W1[b] @ x[b] + b1[b])                      -> [65]
  out      = W2[b] @ hid + b2[b]                             -> [64]

Strategy: G_k = x @ w1k^T computed as shared bf16 matmuls (batch on
partition, fp32 PSUM accumulate); the per-sample k-contraction
sum_k A[b,k]*G_k[b,:] reads PSUM directly: a DVE STT chain covers most
k-terms, ACT scaled-copies produce the rest, and Pool (gpsimd) does the
SBUF-only add tree. Engine-ISA constraints found on real TRN2: gpsimd
cannot access PSUM and TensorScalarPtr is not a valid Pool opcode, so
every scalar MAC must live on DVE or ACT — Pool can only tensor-add.
head_w's bias blocks are folded into the G matmuls via ones-rows
(xA row 64 = 1 from host; hid col 65 = tanh(37) = 1 via preset psum-acc
column), so no separate bias matmuls are needed. Trig features are
computed host-side in float64 and shipped as bf16 (the bf16 matmul
would quantize them identically anyway). All weights ship in one packed
blob: DMA trigger descriptor-generation costs ~0.5us per transfer on the
issuing engine, so fewer, bigger DMAs win.

Sharding: pure data parallel over 8 NeuronCores (2048 rows each).
"""

import sys

sys.path.insert(0, "/opt/trn_rl_repo")

import numpy as np

import concourse.bacc as bacc
import concourse.mybir as mybir
import concourse.tile as tile
from concourse.alu_op_type import AluOpType
from concourse.bass_utils import run_bass_kernel_spmd

DIM = 64
MLPS = DIM + 1          # 65
B = 16384
H = DIM + 2             # 66
P = MLPS * 2 * DIM + MLPS + DIM * MLPS + DIM
O1 = MLPS * 2 * DIM     # 8320  end of W1 block
O2 = O1 + MLPS          # 8385  end of b1 block
O3 = O2 + DIM * MLPS    # 12545 end of W2 block
N_CORES = 8
BS = B // N_CORES       # 2048 rows per core
CH = 512                # phase-A chunk
NCH = BS // CH          # 4
NT = BS // 128          # 16 batch tiles per core

# packed weight blob column offsets (bf16, [128, XB])
B_W0T = 0
B_W1T = B_W0T + H        # 66
B_W2T = B_W1T + H        # 132
B_W3T = B_W2T + H        # 198
B_W1RA = B_W3T + 8       # 206
B_W1RB = B_W1RA + 8 * MLPS   # 726
B_W2RE = B_W1RB + 8 * MLPS   # 1246
B_IDENT = B_W2RE + 8 * DIM   # 1758
XB0 = B_IDENT + 128          # 1886
B_C1FA = XB0                 # flag-only regions
B_C1FB = B_C1FA + MLPS
B_C2F = B_C1FB + MLPS
XB1 = B_C2F + DIM            # 2080

F32 = mybir.dt.float32
BF16 = mybir.dt.bfloat16
NPBF = mybir.dt.np(BF16)
TANH = mybir.ActivationFunctionType.Tanh
COPY = mybir.ActivationFunctionType.Copy

_CACHE: dict = {}


def build_bass(has_hbw: bool, has_hb3: bool):
    nc = bacc.Bacc("TRN2", target_bir_lowering=False, debug=False,
                   num_devices=N_CORES)

    XB = XB1 if has_hbw else XB0
    d_zT = nc.dram_tensor("zT", [MLPS, BS], BF16, kind="ExternalInput")
    d_xA = nc.dram_tensor("xA", [MLPS, BS], BF16, kind="ExternalInput")
    d_xB = nc.dram_tensor("xB", [DIM, BS], BF16, kind="ExternalInput")
    d_wb = nc.dram_tensor("wb", [128, XB], BF16, kind="ExternalInput")
    d_fb = nc.dram_tensor("fb", [H, 3], F32, kind="ExternalInput")
    d_c37 = nc.dram_tensor("c37", [128, 1], F32, kind="ExternalInput")
    d_hb3r = (nc.dram_tensor("hb3r", [128, 64], F32, kind="ExternalInput")
              if has_hb3 else None)
    d_out = nc.dram_tensor("out", [BS, DIM], F32, kind="ExternalOutput")

    mult, add = AluOpType.mult, AluOpType.add
    PSUM = "PSUM"

    with tile.TileContext(nc) as tc:
        with tc.tile_pool(name="const", bufs=1) as cp:
            zT = cp.tile([MLPS, BS], BF16)
            xA = cp.tile([MLPS, BS], BF16)
            xB = cp.tile([DIM, BS], BF16)
            wb = cp.tile([128, XB], BF16)
            fb = cp.tile([H, 3], F32)
            c37 = cp.tile([128, 1], F32)
            hb3r = cp.tile([128, 64], F32) if has_hb3 else None

            w0T = wb[0:MLPS, B_W0T:B_W0T + H]
            w1T = wb[0:H, B_W1T:B_W1T + H]
            w2T = wb[0:H, B_W2T:B_W2T + H]
            w3T = wb[0:H, B_W3T:B_W3T + 8]
            w1rA = wb[0:MLPS, B_W1RA:B_W1RA + 8 * MLPS]
            w1rB = wb[0:DIM, B_W1RB:B_W1RB + 8 * MLPS]
            w2re = wb[0:H, B_W2RE:B_W2RE + 8 * DIM]
            ident = wb[0:128, B_IDENT:B_IDENT + 128]
            hb0, hb1, hb2 = fb[:, 0:1], fb[:, 1:2], fb[:, 2:3]
            c1fA = wb[0:MLPS, B_C1FA:B_C1FA + MLPS] if has_hbw else None
            c1fB = wb[0:DIM, B_C1FB:B_C1FB + MLPS] if has_hbw else None
            c2f = wb[0:H, B_C2F:B_C2F + DIM] if has_hbw else None

            # input loads: 3 parallel DMA queues; per-transfer trigger cost
            # is ~0.5-0.8us on the issuing engine, so few big transfers.
            # Phase-A weights (blob cols 0:B_W1RA) ship first and tiny.
            HB = BS // 2
            nc.scalar.dma_start(zT[:, 0:HB], d_zT[:, 0:HB])
            nc.scalar.dma_start(wb[:, 0:B_W1RA], d_wb[:, 0:B_W1RA])
            nc.sync.dma_start(wb[:, B_W1RA:B_W2RE], d_wb[:, B_W1RA:B_W2RE])
            nc.sync.dma_start(wb[:, B_W2RE:XB], d_wb[:, B_W2RE:XB])
            nc.sync.dma_start(fb[:], d_fb[:])
            nc.sync.dma_start(c37[:], d_c37[:])
            nc.sync.dma_start(zT[:, HB:BS], d_zT[:, HB:BS])
            nc.sync.dma_start(xA[:, HB:BS], d_xA[:, HB:BS])
            nc.sync.dma_start(xB[:, HB:BS], d_xB[:, HB:BS])
            if has_hb3:
                nc.sync.dma_start(hb3r[:], d_hb3r[:])
            nc.gpsimd.dma_start(xA[:, 0:HB], d_xA[:, 0:HB])
            nc.gpsimd.dma_start(xB[:, 0:HB], d_xB[:, 0:HB])

            Ab = cp.tile([128, 8 * NT], F32)     # batch-major A scalars
            accs = [cp.tile([128, H], F32, name=f"acc{i}") for i in range(4)]
            for a_ in accs:
                nc.vector.tensor_copy(a_[:, MLPS:H], c37[:])  # tanh(37)==1

            with (
                tc.tile_pool(name="ha", bufs=2) as ha,
                tc.tile_pool(name="psA", bufs=1, space=PSUM) as psA,
                tc.tile_pool(name="pk", bufs=3, space=PSUM) as pk,
                tc.tile_pool(name="g2", bufs=2, space=PSUM) as g2,
                tc.tile_pool(name="tp", bufs=1, space=PSUM) as tp,
                tc.tile_pool(name="pb", bufs=1, space=PSUM) as pb,
                tc.tile_pool(name="hd", bufs=3) as hd,
                tc.tile_pool(name="ht", bufs=3) as ht,
                tc.tile_pool(name="ob", bufs=7) as ob,
            ):
                # ---- phase A: hypernet (feature-on-partition, bf16) ----
                # emitted layer-by-layer, interleaved with main-loop work:
                # PE queues are in-order, so a monolithic mm->tanh->mm chain
                # would stall every matmul queued behind it.
                pa_state: dict = {}

                def pa_step(c, step):
                    if step == 0:
                        sl = slice(c * CH, (c + 1) * CH)
                        p0 = psA.tile([H, CH], F32, tag="hp")
                        nc.tensor.matmul(p0[:], w0T, zT[:, sl])
                        h0 = ha.tile([H, CH], BF16, tag="h")
                        nc.scalar.activation(h0[:], p0[:], TANH, bias=hb0)
                        pa_state[c] = h0
                    elif step == 1:
                        p1 = psA.tile([H, CH], F32, tag="hp")
                        nc.tensor.matmul(p1[:], w1T, pa_state[c][:])
                        h1 = ha.tile([H, CH], BF16, tag="h")
                        nc.scalar.activation(h1[:], p1[:], TANH, bias=hb1)
                        pa_state[c] = h1
                    elif step == 2:
                        p2 = psA.tile([H, CH], F32, tag="hp")
                        nc.tensor.matmul(p2[:], w2T, pa_state[c][:])
                        h2 = ha.tile([H, CH], BF16, tag="h")
                        nc.scalar.activation(h2[:], p2[:], TANH, bias=hb2)
                        pa_state[c] = h2
                    else:
                        h2 = pa_state.pop(c)
                        pAb = pb.tile([128, 4 * 8], F32, tag="pAb")
                        for j4 in range(4):
                            nc.tensor.matmul(pAb[:, j4 * 8:(j4 + 1) * 8],
                                             h2[:, j4 * 128:(j4 + 1) * 128],
                                             w3T)
                        if has_hb3:
                            nc.vector.tensor_add(pAb[:], pAb[:],
                                                 hb3r[:, 0:4 * 8])
                        nc.scalar.activation(Ab[:, c * 32:(c + 1) * 32],
                                             pAb[:], TANH)

                # ---- main loop, software-pipelined ----
                # stage G: G1 matmuls + cast-copy to SBUF bf16 (DVE)
                # stage C: Pool L1 chain, tanh, transpose, G2, split copy,
                #          Pool L2 chain, out-DMA
                g1sbs = {}
                hids = {}

                def stage_g(j):
                    jsl = slice(j * 128, (j + 1) * 128)
                    pA = pk.tile([128, 4 * MLPS], F32, tag="pk")
                    nc.tensor.matmul(pA[:], xA[:, jsl], w1rA[:, 0:4 * MLPS],
                                     start=True, stop=False)
                    nc.tensor.matmul(pA[:], xB[:, jsl], w1rB[:, 0:4 * MLPS],
                                     start=False, stop=True)
                    pB = pk.tile([128, 4 * MLPS], F32, tag="pk")
                    nc.tensor.matmul(pB[:], xA[:, jsl],
                                     w1rA[:, 4 * MLPS:8 * MLPS],
                                     start=True, stop=False)
                    nc.tensor.matmul(pB[:], xB[:, jsl],
                                     w1rB[:, 4 * MLPS:8 * MLPS],
                                     start=False, stop=True)
                    g1sbs[j] = (pA, pB)

                def stage_c(j):
                    jsl = slice(j * 128, (j + 1) * 128)
                    sA = lambda k: Ab[:, j * 8 + k: j * 8 + k + 1]
                    acc = accs[j % 4]
                    am = acc[:, 0:MLPS]
                    pA, pB = g1sbs.pop(j)
                    gA = lambda k: pA[:, k * MLPS:(k + 1) * MLPS]
                    gB = lambda k: pB[:, k * MLPS:(k + 1) * MLPS]
                    # L1 direct from PSUM. TensorScalarPtr is NOT a valid
                    # Pool opcode on TRN2, so scalar MACs live on DVE (STT
                    # chain) and ACT (scaled copies); Pool does plain adds.
                    nc.vector.tensor_scalar(am, gA(0), sA(0), None, mult)
                    for k in range(1, 4):
                        nc.vector.scalar_tensor_tensor(
                            am, gA(k), sA(k), am, mult, add)
                    lc = [ob.tile([128, MLPS], F32, tag=f"lc{k}",
                                  name=f"lc{k}") for k in range(3)]
                    for k in range(3):
                        nc.scalar.activation(lc[k][:], gB(1 + k), COPY,
                                             bias=0.0, scale=sA(5 + k))
                    nc.vector.scalar_tensor_tensor(
                        am, gB(0), sA(4), am, mult, add)
                    u0 = ob.tile([128, MLPS], F32, tag="u0")
                    nc.gpsimd.tensor_tensor(u0[:], lc[0][:], lc[1][:], add)
                    nc.gpsimd.tensor_tensor(u0[:], u0[:], lc[2][:], add)
                    nc.gpsimd.tensor_tensor(am, am, u0[:], add)
                    if has_hbw:
                        c1p = tp.tile([128, MLPS], F32, tag="c1")
                        nc.tensor.matmul(c1p[:], xA[:, jsl], c1fA,
                                         start=True, stop=False)
                        nc.tensor.matmul(c1p[:], xB[:, jsl], c1fB,
                                         start=False, stop=True)
                        nc.vector.scalar_tensor_tensor(
                            am, c1p[:], 1.0, am, mult, add)

                    hid = hd.tile([128, H], BF16, tag="hid")
                    nc.scalar.activation(hid[:], acc[:], TANH)
                    hids[j] = hid

                def stage_c2(j):
                    jsl = slice(j * 128, (j + 1) * 128)
                    sA = lambda k: Ab[:, j * 8 + k: j * 8 + k + 1]
                    hid = hids.pop(j)
                    tpp = tp.tile([H, 128], BF16, tag="tp")
                    nc.tensor.transpose(tpp[:], hid[:], ident)
                    hidT = ht.tile([H, 128], BF16, tag="hidT")
                    nc.vector.tensor_copy(hidT[:], tpp[:])

                    pG2 = g2.tile([128, 8 * DIM], F32, tag="g2")
                    nc.tensor.matmul(pG2[:], hidT[:], w2re)
                    # L2 direct from PSUM: DVE STT chain k0..k4, ACT
                    # scaled-copy products k5..k7, Pool add tree
                    g2s = lambda k: pG2[:, k * DIM:(k + 1) * DIM]
                    of = ob.tile([128, DIM], F32, tag="of")
                    nc.vector.tensor_scalar(of[:], g2s(0), sA(0), None,
                                            mult)
                    for k in range(1, 5):
                        nc.vector.scalar_tensor_tensor(
                            of[:], g2s(k), sA(k), of[:], mult, add)
                    oc = [ob.tile([128, DIM], F32, tag=f"oc{k}",
                                  name=f"oc{k}") for k in range(3)]
                    for i, k in enumerate(range(5, 8)):
                        nc.scalar.activation(oc[i][:], g2s(k), COPY,
                                             bias=0.0, scale=sA(k))
                    t0 = ob.tile([128, DIM], F32, tag="t0")
                    nc.gpsimd.tensor_tensor(t0[:], oc[0][:], oc[1][:], add)
                    nc.gpsimd.tensor_tensor(t0[:], t0[:], oc[2][:], add)
                    nc.gpsimd.tensor_tensor(of[:], of[:], t0[:], add)
                    if has_hbw:
                        c2p = tp.tile([128, DIM], F32, tag="c2")
                        nc.tensor.matmul(c2p[:], hidT[:], c2f)
                        nc.vector.scalar_tensor_tensor(
                            of[:], c2p[:], 1.0, of[:], mult, add)
                    nc.sync.dma_start(d_out[jsl, :], of[:])

                # emission order = per-engine queue order: phase-A steps are
                # spread between stage_g/stage_c emissions so no engine's
                # in-order queue ever parks on a cross-engine round-trip
                LOOKAHEAD = 5
                pa_step(0, 0)
                pa_step(1, 0)
                stage_g(0)
                pa_step(0, 1)
                stage_g(1)
                pa_step(0, 2)
                stage_g(2)
                pa_step(0, 3)          # Ab chunk 0 (tiles 0-3) ready
                stage_g(3)
                pa_step(1, 1)
                stage_g(4)
                pa_step(1, 2)
                pa_step(1, 3)          # Ab chunk 1 (tiles 4-7)
                pa_plan = {2: (2, 0), 3: (2, 1), 4: (2, 2), 5: (2, 3),
                           6: (3, 0), 7: (3, 1), 8: (3, 2), 9: (3, 3)}
                for j in range(NT):
                    if j in pa_plan:
                        pa_step(*pa_plan[j])
                    if j + LOOKAHEAD < NT:
                        stage_g(j + LOOKAHEAD)
                    stage_c(j)
                    if j > 0:
                        stage_c2(j - 1)
                stage_c2(NT - 1)

    nc.compile()
    return nc


def _prep(inputs):
    f = lambda name: np.ascontiguousarray(
        np.asarray(inputs[name], dtype=np.float32))
    t = float(np.asarray(inputs["t"]))
    y, freqs = f("y"), f("freqs")
    hw0, hb0 = f("hw0"), f("hb0")
    hw1, hb1 = f("hw1"), f("hb1")
    hw2, hb2 = f("hw2"), f("hb2")
    hw3, hb3 = f("hw3"), f("hb3")
    head_w, head_b = f("head_w"), f("head_b")

    C = np.ascontiguousarray

    zT = np.empty((MLPS, B), np.float32)
    zT[0, :] = t - 0.5
    zT[1:, :] = freqs.T

    y64 = y[:, :DIM].astype(np.float64)
    xA = np.empty((MLPS, B), np.float32)
    xA[:DIM, :] = np.cos(y64).T
    xA[DIM, :] = 1.0
    xB = np.asarray(np.sin(y64).T, np.float32)

    # w1 fold: head_w W1-block + b1-w row, split cos/sin feature halves
    hw1b = head_w[:O1].reshape(MLPS, 2 * DIM, 8)        # [h, i, k]
    hw2b = head_w[O2:O3].reshape(DIM, MLPS, 8)          # [o, h, k]

    has_hbw = bool(np.any(head_b))
    has_hb3 = bool(np.any(hb3))
    XB = XB1 if has_hbw else XB0

    wb = np.zeros((128, XB), np.float32)
    wb[0:MLPS, B_W0T:B_W0T + H] = hw0.T
    wb[0:H, B_W1T:B_W1T + H] = hw1.T
    wb[0:H, B_W2T:B_W2T + H] = hw2.T
    wb[0:H, B_W3T:B_W3T + 8] = hw3.T
    wb[0:DIM, B_W1RA:B_W1RA + 8 * MLPS] = \
        hw1b[:, :DIM, :].transpose(1, 2, 0).reshape(DIM, 8 * MLPS)
    wb[DIM, B_W1RA:B_W1RA + 8 * MLPS] = head_w[O1:O2].T.reshape(8 * MLPS)
    wb[0:DIM, B_W1RB:B_W1RB + 8 * MLPS] = \
        hw1b[:, DIM:, :].transpose(1, 2, 0).reshape(DIM, 8 * MLPS)
    wb[0:MLPS, B_W2RE:B_W2RE + 8 * DIM] = \
        hw2b.transpose(1, 2, 0).reshape(MLPS, 8 * DIM)
    wb[MLPS, B_W2RE:B_W2RE + 8 * DIM] = head_w[O3:].T.reshape(8 * DIM)
    wb[0:128, B_IDENT:B_IDENT + 128] = np.eye(128, dtype=np.float32)
    if has_hbw:
        hb1b = head_b[:O1].reshape(MLPS, 2 * DIM)       # [h, i]
        wb[0:DIM, B_C1FA:B_C1FA + MLPS] = hb1b[:, :DIM].T
        wb[DIM, B_C1FA:B_C1FA + MLPS] = head_b[O1:O2]
        wb[0:DIM, B_C1FB:B_C1FB + MLPS] = hb1b[:, DIM:].T
        wb[0:MLPS, B_C2F:B_C2F + DIM] = head_b[O2:O3].reshape(DIM, MLPS).T
        wb[MLPS, B_C2F:B_C2F + DIM] = head_b[O3:]

    fbv = np.stack([hb0, hb1, hb2], axis=1).astype(np.float32)

    shared = {"wb": C(wb).astype(NPBF), "fb": C(fbv),
              "c37": np.full((128, 1), 37.0, np.float32)}
    if has_hb3:
        shared["hb3r"] = C(np.tile(hb3[None, :], (128, 8)))

    zTb = C(zT).astype(NPBF)
    xAb = C(xA).astype(NPBF)
    xBb = C(xB).astype(NPBF)
    in_maps = []
    for c in range(N_CORES):
        sl = slice(c * BS, (c + 1) * BS)
        in_maps.append({
            **shared,
            "zT": C(zTb[:, sl]),
            "xA": C(xAb[:, sl]),
            "xB": C(xBb[:, sl]),
        })
    return in_maps, (has_hbw, has_hb3)


def _run(inputs, trace=False):
    in_maps, flags = _prep(inputs)
    if flags not in _CACHE:
        _CACHE[flags] = build_bass(*flags)
    nc = _CACHE[flags]
    res = run_bass_kernel_spmd(nc, in_maps, core_ids=list(range(N_CORES)),
                               trace=trace)
    out = np.concatenate([r["out"] for r in res.results], axis=0)
    return out, res


def kernel(**inputs) -> np.ndarray:
    out, _ = _run(inputs)
    return out


if __name__ == "__main__":
    rng = np.random.default_rng(0)
    demo = {
        "t": np.float32(0.3),
        "y": rng.standard_normal((B, 2 * DIM), dtype=np.float32),
        "freqs": rng.random((B, DIM), dtype=np.float32),
        "hw0": rng.standard_normal((H, 1 + DIM), dtype=np.float32) * 0.05,
        "hb0": np.zeros(H, np.float32),
        "hw1": rng.standard_normal((H, H), dtype=np.float32) * 0.05,
        "hb1": np.zeros(H, np.float32),
        "hw2": rng.standard_normal((H, H), dtype=np.float32) * 0.05,
        "hb2": np.zeros(H, np.float32),
        "hw3": rng.standard_normal((8, H), dtype=np.float32) * 0.05,
        "hb3": np.zeros(8, np.float32),
        "head_w": rng.standard_normal((P, 8), dtype=np.float32) * 0.05,
        "head_b": np.zeros(P, np.float32),
    }
    out = kernel(**demo)
    print("out", out.shape, out.dtype, float(np.abs(out).max()))
